# revision 22
# baseline (speedup 1.0000x reference)
"""Single-head attention (SEQ=8192, EMBED=2048, HEAD=128) on 8 TRN2 NeuronCores.

Sharding: queries (rows of Q / score matrix) are split 1024 rows per core.
Each core projects K^T first for its own x-shard (fp32r matmuls with
PE-transposed x tiles), then V, and kicks ONE combined AllGather of
(K^T shard | V-natural shard) as early as possible so the collective overlaps
the remaining projection work. Scores are computed directly in transposed
layout [t, sq] (t on partitions), so the attention-weights matrix never needs
an on-chip transpose before the A@V matmul; softmax denominators come from
ones-vector matmuls interleaved into the score loop, and the 1/l scaling is
applied to the final [sq, h] tiles.

kernel(**inputs) takes the FULL unsharded inputs and returns the full output.
"""

import math

import numpy as np

import concourse.bacc as bacc
import concourse.mybir as mybir
import concourse.tile as tile
from concourse.bass_utils import run_bass_kernel_spmd
from concourse.masks import make_identity

SEQ, EMBED, HEAD = 8192, 2048, 128
NCORES = 8
P = 128

F32 = mybir.dt.float32
F32R = mybir.dt.float32r
BF16 = mybir.dt.bfloat16

# Gather/score/attention-weight precision: bf16 halves the AllGather payload
# and the K/V/p SBUF footprint (pt double-buffering) at ~5e-3 relative error;
# float32r keeps ~2e-4.
GATHER_BF16 = True
Id = mybir.ActivationFunctionType.Identity
Exp = mybir.ActivationFunctionType.Exp


def emit(nc, seq=SEQ, embed=EMBED, head=HEAD, ncores=NCORES,
         gather_bf16=None):
    if gather_bf16 is None:
        gather_bf16 = GATHER_BF16
    GDT = BF16 if gather_bf16 else F32R
    assert head == P
    s_loc = seq // ncores          # query rows per core
    e_ch = embed // P              # contraction chunks for the projections
    b_ch = s_loc // P              # 128-row blocks in the local shard
    n_halves = 2 if b_ch >= 2 else 1
    n_half = s_loc // n_halves     # projection matmul free dim (<=512)
    assert n_half <= 512
    sq_g = min(256, s_loc)         # phase-2 query group (matmul free dim)
    n_g = s_loc // sq_g
    n_t = seq // P                 # key/value chunks
    quad = 4 if n_t % 4 == 0 else 1
    scale = 1.0 / math.sqrt(head)

    x = nc.dram_tensor("x", [s_loc, embed], F32, kind="ExternalInput").ap()
    wq = nc.dram_tensor("wq", [embed, head], F32, kind="ExternalInput").ap()
    wk = nc.dram_tensor("wk", [embed, head], F32, kind="ExternalInput").ap()
    wv = nc.dram_tensor("wv", [embed, head], F32, kind="ExternalInput").ap()
    bq = nc.dram_tensor("bq", [head], F32, kind="ExternalInput").ap()
    bk = nc.dram_tensor("bk", [head], F32, kind="ExternalInput").ap()
    bv = nc.dram_tensor("bv", [head], F32, kind="ExternalInput").ap()
    out = nc.dram_tensor("out", [s_loc, head], F32, kind="ExternalOutput").ap()

    with tile.TileContext(nc) as tc:
        with (
            tc.tile_pool(name="consts", bufs=1) as consts,
            tc.tile_pool(name="persist", bufs=1) as persist,
            tc.tile_pool(name="dram", bufs=1, space="DRAM") as dram,
        ):
            ident = consts.tile([P, P], F32)
            make_identity(nc, ident)
            ident_r = consts.tile([P, P], F32R)
            nc.vector.tensor_copy(ident_r[:], ident[:])
            ones_f32 = consts.tile([P, 1], F32)
            nc.vector.memset(ones_f32[:], 1.0)
            ones_col = consts.tile([P, 1], GDT)
            nc.vector.tensor_copy(ones_col[:], ones_f32[:])

            # persistent SBUF across the whole kernel
            qt_sb = persist.tile([P, s_loc], GDT)            # Q^T own shard
            kt_sb = persist.tile([P, n_t, P], GDT)           # K^T full
            v_sb = persist.tile([P, n_t, P], GDT)            # V natural full
            ksz = P * s_loc
            ag_in = dram.tile([2 * ksz], GDT)
            ag_out = dram.tile([ncores * 2 * ksz], GDT, addr_space="Shared")
            warm_in = dram.tile([32], F32)
            warm_out = dram.tile([ncores * 32], F32, addr_space="Shared")
            warm_sb = consts.tile([1, 32], F32)
            nc.vector.memset(warm_sb[:], 0.0)
            nc.sync.dma_start(warm_in.rearrange("(a b) -> a b", a=1),
                              warm_sb[:])
            nc.gpsimd.collective_compute(
                "AllGather", mybir.AluOpType.bypass,
                replica_groups=[list(range(ncores))],
                ins=[warm_in.opt()], outs=[warm_out.opt()])

            # ---------------- Phase 1: project own shard ----------------
            with (
                tc.tile_pool(name="p1", bufs=1) as p1,
                tc.tile_pool(name="p1x", bufs=3) as p1x,
                tc.tile_pool(name="trps", bufs=2, space="PSUM") as trps,
                tc.tile_pool(name="projps", bufs=1, space="PSUM") as projps,
            ):
                x_re = x.rearrange("(b p) (e c) -> e p b c", p=P, c=P)
                x_cols = []
                for e in range(e_ch):
                    x_col = p1x.tile([P, b_ch, P], F32R, tag="xcol", bufs=8,
                                     name=f"x_col{e}")
                    if b_ch >= 4:
                        qb = b_ch // 4
                        for q in range(4):
                            eng = nc.sync if q < 2 else nc.scalar
                            eng.dma_start(
                                x_col[:, q * qb:(q + 1) * qb, :],
                                x_re[e, :, q * qb:(q + 1) * qb, :].bitcast(
                                    F32R))
                    else:
                        nc.sync.dma_start(x_col[:], x_re[e].bitcast(F32R))
                    x_cols.append(x_col)

                wq_sb = p1.tile([P, e_ch, head], F32R)
                wk_sb = p1.tile([P, e_ch, head], F32R)
                wv_sb = p1.tile([P, e_ch, head], F32R)
                for w_sb, w_in in ((wk_sb, wk), (wv_sb, wv), (wq_sb, wq)):
                    nc.sync.dma_start(
                        w_sb[:],
                        w_in.rearrange("(c p) h -> p c h", p=P).bitcast(F32R))
                bq_sb = p1.tile([P, 1], F32)
                bk_sb = p1.tile([P, 1], F32)
                bv_sb = p1.tile([P, 1], F32)
                nc.sync.dma_start(bq_sb[:], bq.unsqueeze(1))
                nc.sync.dma_start(bk_sb[:], bk.unsqueeze(1))
                nc.sync.dma_start(bv_sb[:], bv.unsqueeze(1))

                # transpose all of x once (keep x^T resident for K/V/Q reuse)
                # and accumulate the K projection inside the same loop so the
                # K AllGather can be kicked as early as possible.
                xt_all = p1.tile([P, e_ch, s_loc], F32R)
                half = b_ch // n_halves
                k_ps = [projps.tile([P, n_half], F32, tag=f"proj{h}",
                                    name=f"k_ps{h}")
                        for h in range(n_halves)]
                v_ps = [projps.tile([P, n_half], F32, tag=f"vproj{h}",
                                    name=f"v_ps{h}")
                        for h in range(n_halves)]

                def k_mm(e):
                    for h in range(n_halves):
                        nc.tensor.matmul(
                            k_ps[h][:], wk_sb[:, e, :],
                            xt_all[:, e, h * n_half:(h + 1) * n_half],
                            start=(e == 0), stop=(e == e_ch - 1),
                            skip_group_check=True)
                    for h in range(n_halves):
                        nc.tensor.matmul(
                            v_ps[h][:], wv_sb[:, e, :],
                            xt_all[:, e, h * n_half:(h + 1) * n_half],
                            start=(e == 0), stop=(e == e_ch - 1),
                            skip_group_check=True)

                for e in range(e_ch):
                    x_col = x_cols[e]
                    for h in range(n_halves):
                        tr = trps.tile([P, half, P], F32R, tag="tr")
                        for b in range(half):
                            nc.tensor.transpose(
                                tr[:, b, :], x_col[:, h * half + b, :],
                                ident_r[:])
                        dst = xt_all[:, e, h * n_half:(h + 1) * n_half]
                        if e % 2 == 0:
                            nc.vector.tensor_copy(dst, tr[:])
                        else:
                            nc.scalar.copy(dst, tr[:])
                    if e > 0:
                        k_mm(e - 1)
                k_mm(e_ch - 1)

                kt_loc = p1.tile([P, s_loc], GDT)
                for h in range(n_halves):
                    hsl = slice(h * n_half, (h + 1) * n_half)
                    nc.scalar.activation(kt_loc[:, hsl], k_ps[h][:], Id,
                                         bias=bk_sb[:, 0:1])
                nc.sync.dma_start(
                    ag_in[0:ksz].rearrange("(p s) -> p s", p=P), kt_loc[:])

                def project(w_sb, b_sb, dst_sb):
                    for h in range(n_halves):
                        ps = projps.tile([P, n_half], F32, tag=f"proj{h}")
                        hsl = slice(h * n_half, (h + 1) * n_half)
                        for e in range(e_ch):
                            nc.tensor.matmul(
                                ps[:], w_sb[:, e, :], xt_all[:, e, hsl],
                                start=(e == 0), stop=(e == e_ch - 1))
                        nc.scalar.activation(dst_sb[:, hsl], ps[:], Id,
                                             bias=b_sb[:, 0:1])

                # V (already accumulated in-loop): bias copy + transpose to
                # natural layout
                vt_loc = p1.tile([P, s_loc], F32)
                for h in range(n_halves):
                    hsl = slice(h * n_half, (h + 1) * n_half)
                    nc.scalar.activation(vt_loc[:, hsl], v_ps[h][:], Id,
                                         bias=bv_sb[:, 0:1])
                v_nat = p1.tile([P, b_ch, head], GDT)
                for b in range(b_ch):
                    tr2 = trps.tile([P, P], F32, tag="trv")
                    nc.tensor.transpose(tr2[:],
                                        vt_loc[:, b * P:(b + 1) * P], ident[:])
                    nc.vector.tensor_copy(v_nat[:, b, :], tr2[:])
                nc.sync.dma_start(
                    ag_in[ksz:2 * ksz].rearrange("(b p h) -> p b h",
                                                 p=P, h=head), v_nat[:])
                nc.gpsimd.collective_compute(
                    "AllGather", mybir.AluOpType.bypass,
                    replica_groups=[list(range(ncores))],
                    ins=[ag_in.opt()], outs=[ag_out.opt()])

                # Q last (overlaps the collectives)
                project(wq_sb, bq_sb, qt_sb)

            # unpack gathered K^T / V into SBUF
            hk = max(b_ch // 2, 1)
            for r in range(ncores):
                base = r * 2 * ksz
                k_part = ag_out[base:base + ksz].rearrange(
                    "(p b t) -> p b t", p=P, t=P)
                nc.sync.dma_start(
                    kt_sb[:, r * b_ch:r * b_ch + hk, :], k_part[:, 0:hk, :])
                if b_ch > 1:
                    nc.sync.dma_start(
                        kt_sb[:, r * b_ch + hk:(r + 1) * b_ch, :],
                        k_part[:, hk:, :])
                nc.scalar.dma_start(
                    v_sb[:, r * b_ch:(r + 1) * b_ch, :],
                    ag_out[base + ksz:base + 2 * ksz].rearrange(
                        "(b p h) -> p b h", p=P, h=head))

            # ---------------- Phase 2: attention ----------------
            with (
                tc.tile_pool(name="p2", bufs=1) as p2,
                tc.tile_pool(name="p2s", bufs=2) as p2s,
                tc.tile_pool(name="stps", bufs=3, space="PSUM") as stps,
                tc.tile_pool(name="avps", bufs=1, space="PSUM") as avps,
            ):
                lag = 2 * quad
                n_pairs = n_t // 2 if n_t % 2 == 0 else 0
                pending_tail = []
                for g in range(n_g):
                    qg = qt_sb[:, g * sq_g:(g + 1) * sq_g]
                    pt = p2.tile([P, n_t, sq_g], GDT, tag="pt",
                                 bufs=2 if gather_bf16 else 1)
                    ls = p2.tile([P, n_pairs or 1, sq_g], GDT, tag="ls", bufs=2)
                    l_ps = stps.tile([1, sq_g], F32, tag="lps", bufs=1)
                    ot_ps = avps.tile([P, sq_g], F32, tag="ot")

                    def l_mm(j, n_j, l_ps=l_ps, ls=ls):
                        nc.tensor.matmul(
                            l_ps[:], ones_col[:], ls[:, j, :],
                            start=(j == 0), stop=(j == n_j - 1),
                            skip_group_check=True)

                    def av_mm(c, ot_ps=ot_ps, pt=pt):
                        nc.tensor.matmul(
                            ot_ps[:], v_sb[:, c, :], pt[:, c, :],
                            start=(c == 0), stop=(c == n_t - 1),
                            skip_group_check=True)

                    # scores^T quads -> one wide exp per quad; the AV matmuls
                    # for quad (q-2) interleave to keep the PE dense while ACT
                    # catches up on the exps, and DVE pair-sums the exp'd
                    # chunks so the softmax-denominator ones-matmuls on the PE
                    # are halved.
                    g_lag = n_t if g == 0 else lag
                    for cc in range(0, n_t, quad):
                        st_ps = stps.tile([P, quad, sq_g], F32, tag="st")
                        for k in range(quad):
                            nc.tensor.matmul(
                                st_ps[:, k, :], kt_sb[:, cc + k, :], qg,
                                start=True, stop=True, skip_group_check=True)
                        nc.scalar.activation(pt[:, cc:cc + quad, :], st_ps[:],
                                             Exp, scale=scale)
                        if cc >= g_lag:
                            c0 = cc - g_lag
                            if n_pairs and (c0 // quad) % 2 == 1:
                                nc.vector.tensor_tensor(
                                    ls[:, (c0 - quad) // 2:
                                       (c0 - quad) // 2 + quad, :],
                                    pt[:, c0 - quad:c0, :],
                                    pt[:, c0:c0 + quad, :],
                                    mybir.AluOpType.add)
                            for k in range(quad):
                                av_mm(c0 + k)
                    c0 = n_t - g_lag
                    for cc in range(max(c0, 0), n_t, quad):
                        if n_pairs and (cc // quad) % 2 == 1:
                            nc.vector.tensor_tensor(
                                ls[:, (cc - quad) // 2:
                                   (cc - quad) // 2 + quad, :],
                                pt[:, cc - quad:cc, :],
                                pt[:, cc:cc + quad, :],
                                mybir.AluOpType.add)
                        for k in range(quad):
                            av_mm(cc + k)
                    def tail(g=g, pt=pt, ls=ls, l_ps=l_ps, ot_ps=ot_ps,
                             l_mm=l_mm):
                        if n_pairs:
                            w = n_pairs
                            while w > 1:
                                nc.vector.tensor_tensor(
                                    ls[:, 0:w // 2, :], ls[:, 0:w // 2, :],
                                    ls[:, w // 2:w, :], mybir.AluOpType.add)
                                w //= 2
                            l_mm(0, 1)
                        else:
                            for c in range(n_t):
                                nc.tensor.matmul(
                                    l_ps[:], ones_col[:], pt[:, c, :],
                                    start=(c == 0), stop=(c == n_t - 1),
                                    skip_group_check=True)
                        # 1/l as a per-partition column, then scale +
                        # transpose out
                        l_sb = p2s.tile([1, sq_g], F32, tag="lsb")
                        nc.vector.tensor_copy(l_sb[:], l_ps[:])
                        ot_sb = p2s.tile([P, sq_g], F32, tag="otsb")
                        nc.vector.tensor_copy(ot_sb[:], ot_ps[:])
                        for j in range(sq_g // P):
                            lc_ps = stps.tile([P, 1], F32, tag="st",
                                              name="lc_ps")
                            nc.tensor.transpose(
                                lc_ps[:], l_sb[0:1, j * P:(j + 1) * P],
                                ident[0:1, 0:1])
                            r_col = p2s.tile([P, 1], F32, tag="rcol",
                                             name="r_col")
                            nc.vector.reciprocal(r_col[:], lc_ps[:])
                            o_tr = stps.tile([P, P], F32, tag="st",
                                             name="o_tr")
                            nc.tensor.transpose(
                                o_tr[:], ot_sb[:, j * P:(j + 1) * P],
                                ident[:])
                            o_sb = p2s.tile([P, head], F32, tag="osb",
                                            name="o_sb")
                            nc.vector.tensor_scalar_mul(o_sb[:], o_tr[:],
                                                        r_col[:, 0:1])
                            row0 = g * sq_g + j * P
                            nc.sync.dma_start(out[row0:row0 + P, :], o_sb[:])

                    if pending_tail:
                        pending_tail.pop(0)()
                    pending_tail.append(tail)
                for t in pending_tail:
                    t()
    nc.compile()
    return nc


_CACHE = {}


def _get_nc():
    if "nc" not in _CACHE:
        nc = bacc.Bacc("TRN2", target_bir_lowering=False, debug=False,
                       num_devices=NCORES)
        _CACHE["nc"] = emit(nc)
    return _CACHE["nc"]


def kernel(x, Wq, bq, Wk, bk, Wv, bv):
    x = np.ascontiguousarray(np.asarray(x, dtype=np.float32))
    Wq = np.ascontiguousarray(np.asarray(Wq, dtype=np.float32))
    Wk = np.ascontiguousarray(np.asarray(Wk, dtype=np.float32))
    Wv = np.ascontiguousarray(np.asarray(Wv, dtype=np.float32))
    bq = np.ascontiguousarray(np.asarray(bq, dtype=np.float32))
    bk = np.ascontiguousarray(np.asarray(bk, dtype=np.float32))
    bv = np.ascontiguousarray(np.asarray(bv, dtype=np.float32))
    s_loc = SEQ // NCORES
    in_maps = [
        {
            "x": np.ascontiguousarray(x[c * s_loc:(c + 1) * s_loc]),
            "wq": Wq, "wk": Wk, "wv": Wv,
            "bq": bq, "bk": bk, "bv": bv,
        }
        for c in range(NCORES)
    ]
    res = run_bass_kernel_spmd(_get_nc(), in_maps,
                               core_ids=list(range(NCORES)))
    return np.concatenate(
        [res.results[c]["out"] for c in range(NCORES)], axis=0)


# revision 24
# speedup vs baseline: 1.0135x; 1.0135x over previous
"""Single-head attention (SEQ=8192, EMBED=2048, HEAD=128) on 8 TRN2 NeuronCores.

Sharding: queries (rows of Q / score matrix) are split 1024 rows per core.
Each core projects K^T first for its own x-shard (fp32r matmuls with
PE-transposed x tiles), then V, and kicks ONE combined AllGather of
(K^T shard | V-natural shard) as early as possible so the collective overlaps
the remaining projection work. Scores are computed directly in transposed
layout [t, sq] (t on partitions), so the attention-weights matrix never needs
an on-chip transpose before the A@V matmul; softmax denominators come from
ones-vector matmuls interleaved into the score loop, and the 1/l scaling is
applied to the final [sq, h] tiles.

kernel(**inputs) takes the FULL unsharded inputs and returns the full output.
"""

import math

import numpy as np

import concourse.bacc as bacc
import concourse.mybir as mybir
import concourse.tile as tile
from concourse.bass_utils import run_bass_kernel_spmd
from concourse.masks import make_identity

SEQ, EMBED, HEAD = 8192, 2048, 128
NCORES = 8
P = 128

F32 = mybir.dt.float32
F32R = mybir.dt.float32r
BF16 = mybir.dt.bfloat16

# Gather/score/attention-weight precision: bf16 halves the AllGather payload
# and the K/V/p SBUF footprint (pt double-buffering) at ~5e-3 relative error;
# float32r keeps ~2e-4.
GATHER_BF16 = True
# Host-cast x / weights to bf16 so x^T comes from the DMA xbar transpose
# (2-byte only) instead of ~35us of PE transposes.
X_BF16 = False
Id = mybir.ActivationFunctionType.Identity
Exp = mybir.ActivationFunctionType.Exp


def emit(nc, seq=SEQ, embed=EMBED, head=HEAD, ncores=NCORES,
         gather_bf16=None, x_bf16=None):
    if gather_bf16 is None:
        gather_bf16 = GATHER_BF16
    if x_bf16 is None:
        x_bf16 = X_BF16
    GDT = BF16 if gather_bf16 else F32R
    XDT = BF16 if x_bf16 else F32R
    assert head == P
    s_loc = seq // ncores          # query rows per core
    e_ch = embed // P              # contraction chunks for the projections
    b_ch = s_loc // P              # 128-row blocks in the local shard
    n_halves = 2 if b_ch >= 2 else 1
    n_half = s_loc // n_halves     # projection matmul free dim (<=512)
    assert n_half <= 512
    sq_g = min(256, s_loc)         # phase-2 query group (matmul free dim)
    n_g = s_loc // sq_g
    n_t = seq // P                 # key/value chunks
    quad = 4 if n_t % 4 == 0 else 1
    scale = 1.0 / math.sqrt(head)

    IDT = BF16 if x_bf16 else F32
    x = nc.dram_tensor("x", [s_loc, embed], IDT, kind="ExternalInput").ap()
    wq = nc.dram_tensor("wq", [embed, head], IDT, kind="ExternalInput").ap()
    wk = nc.dram_tensor("wk", [embed, head], IDT, kind="ExternalInput").ap()
    wv = nc.dram_tensor("wv", [embed, head], IDT, kind="ExternalInput").ap()
    bq = nc.dram_tensor("bq", [head], F32, kind="ExternalInput").ap()
    bk = nc.dram_tensor("bk", [head], F32, kind="ExternalInput").ap()
    bv = nc.dram_tensor("bv", [head], F32, kind="ExternalInput").ap()
    out = nc.dram_tensor("out", [s_loc, head], F32, kind="ExternalOutput").ap()

    with tile.TileContext(nc) as tc:
        with (
            tc.tile_pool(name="consts", bufs=1) as consts,
            tc.tile_pool(name="persist", bufs=1) as persist,
            tc.tile_pool(name="dram", bufs=1, space="DRAM") as dram,
        ):
            ident = consts.tile([P, P], F32)
            make_identity(nc, ident)
            ident_r = consts.tile([P, P], F32R)
            nc.vector.tensor_copy(ident_r[:], ident[:])
            ones_f32 = consts.tile([P, 1], F32)
            nc.vector.memset(ones_f32[:], 1.0)
            ones_col = consts.tile([P, 1], GDT)
            nc.vector.tensor_copy(ones_col[:], ones_f32[:])

            # persistent SBUF across the whole kernel
            qt_sb = persist.tile([P, s_loc], GDT)            # Q^T own shard
            kt_sb = persist.tile([P, n_t, P], GDT)           # K^T full
            v_sb = persist.tile([P, n_t, P], GDT)            # V natural full
            ksz = P * s_loc
            ag_in = dram.tile([2 * ksz], GDT)
            ag_out = dram.tile([ncores * 2 * ksz], GDT, addr_space="Shared")
            warm_in = dram.tile([32], F32)
            warm_out = dram.tile([ncores * 32], F32, addr_space="Shared")
            warm_sb = consts.tile([1, 32], F32)
            nc.vector.memset(warm_sb[:], 0.0)
            nc.sync.dma_start(warm_in.rearrange("(a b) -> a b", a=1),
                              warm_sb[:])
            nc.gpsimd.collective_compute(
                "AllGather", mybir.AluOpType.bypass,
                replica_groups=[list(range(ncores))],
                ins=[warm_in.opt()], outs=[warm_out.opt()])

            # ---------------- Phase 1: project own shard ----------------
            with (
                tc.tile_pool(name="p1", bufs=1) as p1,
                tc.tile_pool(name="p1x", bufs=3) as p1x,
                tc.tile_pool(name="trps", bufs=2, space="PSUM") as trps,
                tc.tile_pool(name="projps", bufs=1, space="PSUM") as projps,
            ):
                if not x_bf16:
                    e_pair = 2 if e_ch % 2 == 0 else 1
                    x_re = x.rearrange("(b p) (g c) -> g p b c",
                                       p=P, c=P * e_pair)
                    x_cols = []
                    for gi in range(e_ch // e_pair):
                        x_col = p1x.tile([P, b_ch, e_pair * P], F32R,
                                         tag="xcol", bufs=4,
                                         name=f"x_col{gi}")
                        if b_ch >= 4:
                            qb = b_ch // 4
                            for q in range(4):
                                eng = nc.sync if q < 2 else nc.scalar
                                eng.dma_start(
                                    x_col[:, q * qb:(q + 1) * qb, :],
                                    x_re[gi, :, q * qb:(q + 1) * qb,
                                         :].bitcast(F32R))
                        else:
                            nc.sync.dma_start(x_col[:],
                                              x_re[gi].bitcast(F32R))
                        x_cols.append(x_col)

                wq_sb = p1.tile([P, e_ch, head], XDT)
                wk_sb = p1.tile([P, e_ch, head], XDT)
                wv_sb = p1.tile([P, e_ch, head], XDT)
                for w_sb, w_in in ((wk_sb, wk), (wv_sb, wv), (wq_sb, wq)):
                    w_src = w_in.rearrange("(c p) h -> p c h", p=P)
                    nc.sync.dma_start(
                        w_sb[:], w_src if x_bf16 else w_src.bitcast(F32R))
                bq_sb = p1.tile([P, 1], F32)
                bk_sb = p1.tile([P, 1], F32)
                bv_sb = p1.tile([P, 1], F32)
                nc.sync.dma_start(bq_sb[:], bq.unsqueeze(1))
                nc.sync.dma_start(bk_sb[:], bk.unsqueeze(1))
                nc.sync.dma_start(bv_sb[:], bv.unsqueeze(1))

                # transpose all of x once (keep x^T resident for K/V/Q reuse)
                # and accumulate the K projection inside the same loop so the
                # K AllGather can be kicked as early as possible.
                xt_all = p1.tile([P, e_ch, s_loc], XDT)
                half = b_ch // n_halves
                k_ps = [projps.tile([P, n_half], F32, tag=f"proj{h}",
                                    name=f"k_ps{h}")
                        for h in range(n_halves)]
                v_ps = [projps.tile([P, n_half], F32, tag=f"vproj{h}",
                                    name=f"v_ps{h}")
                        for h in range(n_halves)]

                def k_mm(e):
                    for h in range(n_halves):
                        nc.tensor.matmul(
                            k_ps[h][:], wk_sb[:, e, :],
                            xt_all[:, e, h * n_half:(h + 1) * n_half],
                            start=(e == 0), stop=(e == e_ch - 1),
                            skip_group_check=True)
                    for h in range(n_halves):
                        nc.tensor.matmul(
                            v_ps[h][:], wv_sb[:, e, :],
                            xt_all[:, e, h * n_half:(h + 1) * n_half],
                            start=(e == 0), stop=(e == e_ch - 1),
                            skip_group_check=True)

                if x_bf16:
                    for e in range(e_ch):
                        eng = nc.sync if e % 2 == 0 else nc.scalar
                        eng.dma_start_transpose(
                            xt_all[:, e, :], x[:, e * P:(e + 1) * P])
                        if e > 0:
                            k_mm(e - 1)
                    k_mm(e_ch - 1)
                else:
                    for e in range(e_ch):
                        x_col = x_cols[e // e_pair]
                        ec = (e % e_pair) * P
                        for h in range(n_halves):
                            tr = trps.tile([P, half, P], F32R, tag="tr")
                            for b in range(half):
                                nc.tensor.transpose(
                                    tr[:, b, :],
                                    x_col[:, h * half + b, ec:ec + P],
                                    ident_r[:])
                            dst = xt_all[:, e, h * n_half:(h + 1) * n_half]
                            if e % 2 == 0:
                                nc.vector.tensor_copy(dst, tr[:])
                            else:
                                nc.scalar.copy(dst, tr[:])
                        if e > 0:
                            k_mm(e - 1)
                    k_mm(e_ch - 1)

                kt_loc = p1.tile([P, s_loc], GDT)
                for h in range(n_halves):
                    hsl = slice(h * n_half, (h + 1) * n_half)
                    nc.scalar.activation(kt_loc[:, hsl], k_ps[h][:], Id,
                                         bias=bk_sb[:, 0:1])
                nc.sync.dma_start(
                    ag_in[0:ksz].rearrange("(p s) -> p s", p=P), kt_loc[:])

                def project(w_sb, b_sb, dst_sb):
                    for h in range(n_halves):
                        ps = projps.tile([P, n_half], F32, tag=f"proj{h}")
                        hsl = slice(h * n_half, (h + 1) * n_half)
                        for e in range(e_ch):
                            nc.tensor.matmul(
                                ps[:], w_sb[:, e, :], xt_all[:, e, hsl],
                                start=(e == 0), stop=(e == e_ch - 1))
                        nc.scalar.activation(dst_sb[:, hsl], ps[:], Id,
                                             bias=b_sb[:, 0:1])

                # V (already accumulated in-loop): bias copy + transpose to
                # natural layout
                vt_loc = p1.tile([P, s_loc], F32)
                for h in range(n_halves):
                    hsl = slice(h * n_half, (h + 1) * n_half)
                    nc.scalar.activation(vt_loc[:, hsl], v_ps[h][:], Id,
                                         bias=bv_sb[:, 0:1])
                v_nat = p1.tile([P, b_ch, head], GDT)
                for b in range(b_ch):
                    tr2 = trps.tile([P, P], F32, tag="trv")
                    nc.tensor.transpose(tr2[:],
                                        vt_loc[:, b * P:(b + 1) * P], ident[:])
                    nc.vector.tensor_copy(v_nat[:, b, :], tr2[:])
                nc.sync.dma_start(
                    ag_in[ksz:2 * ksz].rearrange("(b p h) -> p b h",
                                                 p=P, h=head), v_nat[:])
                nc.gpsimd.collective_compute(
                    "AllGather", mybir.AluOpType.bypass,
                    replica_groups=[list(range(ncores))],
                    ins=[ag_in.opt()], outs=[ag_out.opt()])

                # Q last (overlaps the collectives)
                project(wq_sb, bq_sb, qt_sb)

            # unpack gathered K^T / V into SBUF
            hk = max(b_ch // 2, 1)
            for r in range(ncores):
                base = r * 2 * ksz
                k_part = ag_out[base:base + ksz].rearrange(
                    "(p b t) -> p b t", p=P, t=P)
                nc.sync.dma_start(
                    kt_sb[:, r * b_ch:r * b_ch + hk, :], k_part[:, 0:hk, :])
                if b_ch > 1:
                    nc.sync.dma_start(
                        kt_sb[:, r * b_ch + hk:(r + 1) * b_ch, :],
                        k_part[:, hk:, :])
                nc.scalar.dma_start(
                    v_sb[:, r * b_ch:(r + 1) * b_ch, :],
                    ag_out[base + ksz:base + 2 * ksz].rearrange(
                        "(b p h) -> p b h", p=P, h=head))

            # ---------------- Phase 2: attention ----------------
            with (
                tc.tile_pool(name="p2", bufs=1) as p2,
                tc.tile_pool(name="p2s", bufs=2) as p2s,
                tc.tile_pool(name="stps", bufs=3, space="PSUM") as stps,
                tc.tile_pool(name="avps", bufs=1, space="PSUM") as avps,
            ):
                lag = 2 * quad
                n_pairs = n_t // 2 if n_t % 2 == 0 else 0
                pending_tail = []
                for g in range(n_g):
                    qg = qt_sb[:, g * sq_g:(g + 1) * sq_g]
                    pt = p2.tile([P, n_t, sq_g], GDT, tag="pt",
                                 bufs=2 if gather_bf16 else 1)
                    ls = p2.tile([P, n_pairs or 1, sq_g], GDT, tag="ls", bufs=2)
                    l_ps = stps.tile([1, sq_g], F32, tag="lps", bufs=1)
                    ot_ps = avps.tile([P, sq_g], F32, tag="ot")

                    def l_mm(j, n_j, l_ps=l_ps, ls=ls):
                        nc.tensor.matmul(
                            l_ps[:], ones_col[:], ls[:, j, :],
                            start=(j == 0), stop=(j == n_j - 1),
                            skip_group_check=True)

                    def av_mm(c, ot_ps=ot_ps, pt=pt):
                        nc.tensor.matmul(
                            ot_ps[:], v_sb[:, c, :], pt[:, c, :],
                            start=(c == 0), stop=(c == n_t - 1),
                            skip_group_check=True)

                    # scores^T quads -> one wide exp per quad; the AV matmuls
                    # for quad (q-2) interleave to keep the PE dense while ACT
                    # catches up on the exps, and DVE pair-sums the exp'd
                    # chunks so the softmax-denominator ones-matmuls on the PE
                    # are halved.
                    g_lag = n_t if g == 0 else lag
                    for cc in range(0, n_t, quad):
                        st_ps = stps.tile([P, quad, sq_g], F32, tag="st")
                        for k in range(quad):
                            nc.tensor.matmul(
                                st_ps[:, k, :], kt_sb[:, cc + k, :], qg,
                                start=True, stop=True, skip_group_check=True)
                        nc.scalar.activation(pt[:, cc:cc + quad, :], st_ps[:],
                                             Exp, scale=scale)
                        if cc >= g_lag:
                            c0 = cc - g_lag
                            if n_pairs and (c0 // quad) % 2 == 1:
                                nc.vector.tensor_tensor(
                                    ls[:, (c0 - quad) // 2:
                                       (c0 - quad) // 2 + quad, :],
                                    pt[:, c0 - quad:c0, :],
                                    pt[:, c0:c0 + quad, :],
                                    mybir.AluOpType.add)
                            for k in range(quad):
                                av_mm(c0 + k)
                    c0 = n_t - g_lag
                    for cc in range(max(c0, 0), n_t, quad):
                        if n_pairs and (cc // quad) % 2 == 1:
                            nc.vector.tensor_tensor(
                                ls[:, (cc - quad) // 2:
                                   (cc - quad) // 2 + quad, :],
                                pt[:, cc - quad:cc, :],
                                pt[:, cc:cc + quad, :],
                                mybir.AluOpType.add)
                        for k in range(quad):
                            av_mm(cc + k)
                    def tail(g=g, pt=pt, ls=ls, l_ps=l_ps, ot_ps=ot_ps,
                             l_mm=l_mm):
                        if n_pairs:
                            w = n_pairs
                            while w > 1:
                                nc.vector.tensor_tensor(
                                    ls[:, 0:w // 2, :], ls[:, 0:w // 2, :],
                                    ls[:, w // 2:w, :], mybir.AluOpType.add)
                                w //= 2
                            l_mm(0, 1)
                        else:
                            for c in range(n_t):
                                nc.tensor.matmul(
                                    l_ps[:], ones_col[:], pt[:, c, :],
                                    start=(c == 0), stop=(c == n_t - 1),
                                    skip_group_check=True)
                        # 1/l as a per-partition column, then scale +
                        # transpose out
                        l_sb = p2s.tile([1, sq_g], F32, tag="lsb")
                        nc.vector.tensor_copy(l_sb[:], l_ps[:])
                        ot_sb = p2s.tile([P, sq_g], F32, tag="otsb")
                        nc.vector.tensor_copy(ot_sb[:], ot_ps[:])
                        for j in range(sq_g // P):
                            lc_ps = stps.tile([P, 1], F32, tag="st",
                                              name="lc_ps")
                            nc.tensor.transpose(
                                lc_ps[:], l_sb[0:1, j * P:(j + 1) * P],
                                ident[0:1, 0:1])
                            r_col = p2s.tile([P, 1], F32, tag="rcol",
                                             name="r_col")
                            nc.vector.reciprocal(r_col[:], lc_ps[:])
                            o_tr = stps.tile([P, P], F32, tag="st",
                                             name="o_tr")
                            nc.tensor.transpose(
                                o_tr[:], ot_sb[:, j * P:(j + 1) * P],
                                ident[:])
                            o_sb = p2s.tile([P, head], F32, tag="osb",
                                            name="o_sb")
                            nc.vector.tensor_scalar_mul(o_sb[:], o_tr[:],
                                                        r_col[:, 0:1])
                            row0 = g * sq_g + j * P
                            nc.sync.dma_start(out[row0:row0 + P, :], o_sb[:])

                    if pending_tail:
                        pending_tail.pop(0)()
                    pending_tail.append(tail)
                for t in pending_tail:
                    t()
    nc.compile()
    return nc


_CACHE = {}


def _get_nc():
    if "nc" not in _CACHE:
        nc = bacc.Bacc("TRN2", target_bir_lowering=False, debug=False,
                       num_devices=NCORES)
        _CACHE["nc"] = emit(nc)
    return _CACHE["nc"]


def kernel(x, Wq, bq, Wk, bk, Wv, bv):
    if X_BF16:
        import ml_dtypes
        mdt = ml_dtypes.bfloat16
    else:
        mdt = np.float32
    x = np.ascontiguousarray(np.asarray(x, dtype=np.float32).astype(mdt))
    Wq = np.ascontiguousarray(np.asarray(Wq, dtype=np.float32).astype(mdt))
    Wk = np.ascontiguousarray(np.asarray(Wk, dtype=np.float32).astype(mdt))
    Wv = np.ascontiguousarray(np.asarray(Wv, dtype=np.float32).astype(mdt))
    bq = np.ascontiguousarray(np.asarray(bq, dtype=np.float32))
    bk = np.ascontiguousarray(np.asarray(bk, dtype=np.float32))
    bv = np.ascontiguousarray(np.asarray(bv, dtype=np.float32))
    s_loc = SEQ // NCORES
    in_maps = [
        {
            "x": np.ascontiguousarray(x[c * s_loc:(c + 1) * s_loc]),
            "wq": Wq, "wk": Wk, "wv": Wv,
            "bq": bq, "bk": bk, "bv": bv,
        }
        for c in range(NCORES)
    ]
    res = run_bass_kernel_spmd(_get_nc(), in_maps,
                               core_ids=list(range(NCORES)))
    return np.concatenate(
        [res.results[c]["out"] for c in range(NCORES)], axis=0)
